# revision 6
# baseline (speedup 1.0000x reference)
"""Multi-head attention (B=2, S=2048, D=512, H=8) on 8 trn2 NeuronCores.

Sharding: data-parallel over batch (2) x tensor-parallel over head-pairs (4).
Core c handles batch c//4 and heads [2*(c%4), 2*(c%4)+1] (128 model dims).

Device kernel (SPMD, identical program, per-core inputs):
  inputs:  xqT/xkT/xvT [512,2048] (host-pretransposed), wq/wk/wv [512,128]
           (column slice), wo [128,512] (row slice), bq/bk [128,1]
  outputs: attn_out [2,2048,2048] (this core's two heads, softmaxed),
           out_partial [2048,512] (this core's contribution to out)

Host folds the v/o biases in afterwards: since each softmax row sums to 1,
ctx = attn@(vh + 1*bv^T) = attn@vh + bv, so out += bv @ wo_w + wo_b.
"""

import numpy as np

import concourse.bass as bass
import concourse.mybir as mybir
from concourse import bacc
from concourse.tile import TileContext
from concourse.bass_utils import run_bass_kernel_spmd

B, S, D = 2, 2048, 512
H, DEP = 8, 64
N_CORES = 8
HPC = 2          # heads per core
D2 = HPC * DEP   # 128 model dims per core
NT = S // 128    # 16 tiles of 128 rows
F32 = mybir.dt.float32
F16 = mybir.dt.float16

_CACHED_NC = None


def _build_nc():
    nc = bacc.Bacc(None, target_bir_lowering=False, debug=False)

    xqT = nc.declare_dram_parameter("xqT", [D, S], F32, isOutput=False)
    xkT = nc.declare_dram_parameter("xkT", [D, S], F32, isOutput=False)
    xvT = nc.declare_dram_parameter("xvT", [D, S], F32, isOutput=False)
    wq = nc.declare_dram_parameter("wq", [D, D2], F32, isOutput=False)
    wk = nc.declare_dram_parameter("wk", [D, D2], F32, isOutput=False)
    wv = nc.declare_dram_parameter("wv", [D, D2], F32, isOutput=False)
    wo = nc.declare_dram_parameter("wo", [D2, D], F32, isOutput=False)
    bq = nc.declare_dram_parameter("bq", [D2, 1], F32, isOutput=False)
    bk = nc.declare_dram_parameter("bk", [D2, 1], F32, isOutput=False)
    attn_out = nc.declare_dram_parameter("attn_out", [HPC, S, S], F32, isOutput=True)
    out_partial = nc.declare_dram_parameter("out_partial", [S, D], F32, isOutput=True)

    scale = 1.0 / np.sqrt(DEP)

    with TileContext(nc) as tc:
        with tc.tile_pool(name="singles", bufs=1) as singles:
            # persistent SBUF tensors
            qhT = singles.tile([D2, S], F16, tag="qhT")     # [128, 2048]
            khT = singles.tile([D2, S], F16, tag="khT")
            vh = singles.tile([128, S], F16, tag="vh")     # block st: [128(k),128(d)]
            ctxT = singles.tile([D2, S], F32, tag="ctxT")   # [128(d), 2048(q)] unnormalized
            recip = singles.tile([128, HPC * NT], F32, tag="recip")  # col h*16+qt
            wq_sb = singles.tile([128, 4, D2], F32, tag="wq")
            wk_sb = singles.tile([128, 4, D2], F32, tag="wk")
            wv_sb = singles.tile([128, 4, D2], F32, tag="wv")
            wo_sb = singles.tile([D2, D], F32, tag="wo")
            bq_sb = singles.tile([D2, 1], F32, tag="bq")
            bk_sb = singles.tile([D2, 1], F32, tag="bk")

            nc.sync.dma_start(out=wq_sb[:], in_=wq.ap().rearrange("(c p) d -> p c d", p=128))
            nc.sync.dma_start(out=wk_sb[:], in_=wk.ap().rearrange("(c p) d -> p c d", p=128))
            nc.sync.dma_start(out=wv_sb[:], in_=wv.ap().rearrange("(c p) d -> p c d", p=128))
            nc.sync.dma_start(out=wo_sb[:], in_=wo.ap())
            nc.sync.dma_start(out=bq_sb[:], in_=bq.ap())
            nc.sync.dma_start(out=bk_sb[:], in_=bk.ap())

            # ---------------- stage 1: projections ----------------
            HW = S // 2  # 1024-col halves of the sequence
            with (
                tc.tile_pool(name="xpool", bufs=2) as xpool,
                tc.tile_pool(name="psum1", bufs=2, space="PSUM") as psum1,
            ):
                # qhT = wq.T @ xqT (+bq), khT likewise
                for name, xT, w_sb, b_sb, outT in (
                    ("q", xqT, wq_sb, bq_sb, qhT),
                    ("k", xkT, wk_sb, bk_sb, khT),
                ):
                    for half in range(2):
                        x_sb = xpool.tile([128, 4, HW], F32, tag="x")
                        nc.sync.dma_start(
                            out=x_sb[:],
                            in_=xT.ap().rearrange("(c p) s -> p c s", p=128)[
                                :, :, half * HW : (half + 1) * HW
                            ],
                        )
                        ps = psum1.tile([D2, HW], F32, tag="ps1")
                        for ncx in range(HW // 512):
                            for cc in range(4):
                                nc.tensor.matmul(
                                    ps[:, ncx * 512 : (ncx + 1) * 512],
                                    w_sb[:, cc, :],
                                    x_sb[:, cc, ncx * 512 : (ncx + 1) * 512],
                                    start=(cc == 0),
                                    stop=(cc == 3),
                                )
                        nc.vector.tensor_scalar_add(
                            outT[:, half * HW : (half + 1) * HW], ps[:], b_sb[:]
                        )
                # vh[st block] = (xv @ wv) rows; bias folded on host
                for half in range(2):
                    x_sb = xpool.tile([128, 4, HW], F32, tag="x")
                    nc.sync.dma_start(
                        out=x_sb[:],
                        in_=xvT.ap().rearrange("(c p) s -> p c s", p=128)[
                            :, :, half * HW : (half + 1) * HW
                        ],
                    )
                    ps = psum1.tile([128, HW], F32, tag="ps1")
                    for sti in range(HW // 128):
                        for cc in range(4):
                            nc.tensor.matmul(
                                ps[:, sti * 128 : (sti + 1) * 128],
                                x_sb[:, cc, sti * 128 : (sti + 1) * 128],
                                wv_sb[:, cc, :],
                                start=(cc == 0),
                                stop=(cc == 3),
                            )
                    nc.vector.tensor_copy(
                        out=vh[:, half * HW : (half + 1) * HW], in_=ps[:]
                    )

            # ---------------- stage 2: attention ----------------
            with (
                tc.tile_pool(name="psum_p", bufs=1, space="PSUM") as pool_p,
                tc.tile_pool(name="psum_pt", bufs=1, space="PSUM") as pool_pt,
                tc.tile_pool(name="psum_av", bufs=2, space="PSUM") as pool_av,
                tc.tile_pool(name="Ppool", bufs=3) as Ppool,
                tc.tile_pool(name="stpool", bufs=2) as stpool,
                tc.tile_pool(name="PTpool", bufs=1) as PTpool,
                tc.tile_pool(name="sumpool", bufs=4) as sumpool,
            ):
                for h in range(HPC):
                    hs = slice(h * DEP, (h + 1) * DEP)
                    PT = PTpool.tile([128, NT, S], F16, tag="PT")
                    for t in range(NT):
                        # ---- P stream (q-tile t): attn rows out ----
                        ps_p = pool_p.tile([128, S], F32, tag="pp")
                        for ncx in range(4):
                            nc.tensor.matmul(
                                ps_p[:, ncx * 512 : (ncx + 1) * 512],
                                qhT[hs, t * 128 : (t + 1) * 128],
                                khT[hs, ncx * 512 : (ncx + 1) * 512],
                                start=True,
                                stop=True,
                            )
                        P_sb = Ppool.tile([128, S], F32, tag="P")
                        sums = sumpool.tile([128, 1], F32, tag="sums")
                        nc.scalar.activation(
                            P_sb[:],
                            ps_p[:],
                            mybir.ActivationFunctionType.Exp,
                            scale=float(scale),
                            accum_out=sums[:],
                        )
                        rc = recip[:, h * NT + t : h * NT + t + 1]
                        nc.vector.reciprocal(rc, sums[:])
                        nc.gpsimd.tensor_scalar_mul(P_sb[:], P_sb[:], rc)
                        nc.sync.dma_start(
                            out=attn_out[h, t * 128 : (t + 1) * 128, :], in_=P_sb[:]
                        )
                        # ---- PT stream (k-tile t): transposed probs ----
                        st_sb = stpool.tile([128, S], F32, tag="st")
                        for half in range(2):
                            ps_t = pool_pt.tile([128, 1024], F32, tag="pt")
                            for ncx in range(2):
                                nc.tensor.matmul(
                                    ps_t[:, ncx * 512 : (ncx + 1) * 512],
                                    khT[hs, t * 128 : (t + 1) * 128],
                                    qhT[hs, half * 1024 + ncx * 512 : half * 1024 + (ncx + 1) * 512],
                                    start=True,
                                    stop=True,
                                )
                            nc.vector.tensor_copy(
                                out=st_sb[:, half * 1024 : (half + 1) * 1024],
                                in_=ps_t[:],
                            )
                        nc.scalar.activation(
                            PT[:, t, :],
                            st_sb[:],
                            mybir.ActivationFunctionType.Exp,
                            scale=float(scale),
                        )
                    # ---- AV: ctxT[h] += vh_h[kt].T @ PT[kt] over kt ----
                    for qc in range(4):
                        ps_av = pool_av.tile([DEP, 512], F32, tag="av")
                        for kt in range(NT):
                            nc.tensor.matmul(
                                ps_av[:],
                                vh[:, kt * 128 + h * DEP : kt * 128 + (h + 1) * DEP],
                                PT[:, kt, qc * 512 : (qc + 1) * 512],
                                start=(kt == 0),
                                stop=(kt == NT - 1),
                            )
                        nc.vector.tensor_copy(
                            out=ctxT[hs, qc * 512 : (qc + 1) * 512], in_=ps_av[:]
                        )

            # ---------------- stage 3: output projection ----------------
            with (
                tc.tile_pool(name="psum_o", bufs=4, space="PSUM") as pool_o,
                tc.tile_pool(name="opool", bufs=3) as opool,
            ):
                for t in range(NT):
                    acc = None
                    for h in range(HPC):
                        hs = slice(h * DEP, (h + 1) * DEP)
                        ps_o = pool_o.tile([128, D], F32, tag="po")
                        nc.tensor.matmul(
                            ps_o[:],
                            ctxT[hs, t * 128 : (t + 1) * 128],
                            wo_sb[hs, :],
                            start=True,
                            stop=True,
                        )
                        tmp = opool.tile([128, D], F32, tag=f"otmp{h}")
                        nc.vector.tensor_scalar_mul(
                            tmp[:], ps_o[:], recip[:, h * NT + t : h * NT + t + 1]
                        )
                        if acc is None:
                            acc = tmp
                        else:
                            nc.vector.tensor_add(acc[:], acc[:], tmp[:])
                    nc.sync.dma_start(
                        out=out_partial[t * 128 : (t + 1) * 128, :], in_=acc[:]
                    )

    nc.finalize()
    return nc


def kernel(q, k, v, wq_w, wq_b, wk_w, wk_b, wv_w, wv_b, wo_w, wo_b, _profile=False):
    global _CACHED_NC
    q = np.asarray(q, np.float32)
    k = np.asarray(k, np.float32)
    v = np.asarray(v, np.float32)
    wq_w = np.asarray(wq_w, np.float32)
    wk_w = np.asarray(wk_w, np.float32)
    wv_w = np.asarray(wv_w, np.float32)
    wo_w = np.asarray(wo_w, np.float32)

    if _CACHED_NC is None:
        _CACHED_NC = _build_nc()
    nc = _CACHED_NC

    xT = {}
    for b in range(B):
        xT[("q", b)] = np.ascontiguousarray(q[b].T)
        xT[("k", b)] = np.ascontiguousarray(k[b].T)
        xT[("v", b)] = np.ascontiguousarray(v[b].T)

    in_maps = []
    for c in range(N_CORES):
        b, hp = divmod(c, 4)
        cs = slice(hp * D2, (hp + 1) * D2)
        in_maps.append(
            {
                "xqT": xT[("q", b)],
                "xkT": xT[("k", b)],
                "xvT": xT[("v", b)],
                "wq": np.ascontiguousarray(wq_w[:, cs]),
                "wk": np.ascontiguousarray(wk_w[:, cs]),
                "wv": np.ascontiguousarray(wv_w[:, cs]),
                "wo": np.ascontiguousarray(wo_w[cs, :]),
                "bq": np.ascontiguousarray(np.asarray(wq_b, np.float32)[cs, None]),
                "bk": np.ascontiguousarray(np.asarray(wk_b, np.float32)[cs, None]),
            }
        )

    kwargs = {}
    if _profile:
        import os

        os.makedirs("/tmp/bass_trace", exist_ok=True)
        kwargs = {"trace": True, "tmpdir": "/tmp/bass_trace"}
    res = run_bass_kernel_spmd(nc, in_maps, list(range(N_CORES)), **kwargs)

    attn = np.empty((B, H, S, S), np.float32)
    out = np.zeros((B, S, D), np.float32)
    for c in range(N_CORES):
        b, hp = divmod(c, 4)
        attn[b, 2 * hp : 2 * hp + 2] = res.results[c]["attn_out"]
        out[b] += res.results[c]["out_partial"]
    # fold v/o biases: softmax rows sum to 1 -> ctx += wv_b, out += wv_b@wo + wo_b
    out += (
        np.asarray(wv_b, np.float32) @ wo_w + np.asarray(wo_b, np.float32)
    )[None, None, :]

    if _profile:
        return (out, attn), res
    return out, attn


# revision 7
# speedup vs baseline: 4.1824x; 4.1824x over previous
"""Multi-head attention (B=2, S=2048, D=512, H=8) on 8 trn2 NeuronCores.

Sharding: data-parallel over batch (2) x tensor-parallel over head-pairs (4).
Core c handles batch c//4 and heads [2*(c%4), 2*(c%4)+1] (128 model dims).

Device kernel (SPMD, identical program, per-core inputs):
  inputs:  xqT/xkT/xvT [512,2048] (host-pretransposed), wq/wk/wv [512,128]
           (column slice), wo [128,512] (row slice), bq/bk [128,1]
  outputs: attn_out [2,2048,2048] (this core's two heads, softmaxed),
           out_partial [2048,512] (this core's contribution to out)

Host folds the v/o biases in afterwards: since each softmax row sums to 1,
ctx = attn@(vh + 1*bv^T) = attn@vh + bv, so out += bv @ wo_w + wo_b.
"""

import numpy as np

import concourse.bass as bass
import concourse.mybir as mybir
from concourse import bacc
from concourse.tile import TileContext
from concourse.bass_utils import run_bass_kernel_spmd

B, S, D = 2, 2048, 512
H, DEP = 8, 64
N_CORES = 8
HPC = 2          # heads per core
D2 = HPC * DEP   # 128 model dims per core
NT = S // 128    # 16 tiles of 128 rows
F32 = mybir.dt.float32
F16 = mybir.dt.float16

_CACHED_NC = None


def _build_nc():
    nc = bacc.Bacc(None, target_bir_lowering=False, debug=False)

    xqT = nc.declare_dram_parameter("xqT", [D, S], F32, isOutput=False)
    xkT = nc.declare_dram_parameter("xkT", [D, S], F32, isOutput=False)
    xvT = nc.declare_dram_parameter("xvT", [D, S], F32, isOutput=False)
    wq = nc.declare_dram_parameter("wq", [D, D2], F32, isOutput=False)
    wk = nc.declare_dram_parameter("wk", [D, D2], F32, isOutput=False)
    wv = nc.declare_dram_parameter("wv", [D, D2], F32, isOutput=False)
    wo = nc.declare_dram_parameter("wo", [D2, D], F32, isOutput=False)
    bq = nc.declare_dram_parameter("bq", [D2, 1], F32, isOutput=False)
    bk = nc.declare_dram_parameter("bk", [D2, 1], F32, isOutput=False)
    attn_out = nc.declare_dram_parameter("attn_out", [HPC, S, S], F32, isOutput=True)
    out_partial = nc.declare_dram_parameter("out_partial", [S, D], F32, isOutput=True)

    scale = 1.0 / np.sqrt(DEP)

    with TileContext(nc) as tc:
        with tc.tile_pool(name="singles", bufs=1) as singles:
            # persistent SBUF tensors
            qhT = singles.tile([D2, S], F16, tag="qhT")     # [128, 2048]
            khT = singles.tile([D2, S], F16, tag="khT")
            vh = singles.tile([128, S], F16, tag="vh")     # block st: [128(k),128(d)]
            ctxT = singles.tile([D2, S], F32, tag="ctxT")   # [128(d), 2048(q)] unnormalized
            recip = singles.tile([128, HPC * NT], F32, tag="recip")  # col h*16+qt
            wq_sb = singles.tile([128, 4, D2], F32, tag="wq")
            wk_sb = singles.tile([128, 4, D2], F32, tag="wk")
            wv_sb = singles.tile([128, 4, D2], F32, tag="wv")
            wo_sb = singles.tile([D2, D], F32, tag="wo")
            bq_sb = singles.tile([D2, 1], F32, tag="bq")
            bk_sb = singles.tile([D2, 1], F32, tag="bk")

            nc.sync.dma_start(out=wq_sb[:], in_=wq.ap().rearrange("(c p) d -> p c d", p=128))
            nc.sync.dma_start(out=wk_sb[:], in_=wk.ap().rearrange("(c p) d -> p c d", p=128))
            nc.sync.dma_start(out=wv_sb[:], in_=wv.ap().rearrange("(c p) d -> p c d", p=128))
            nc.sync.dma_start(out=wo_sb[:], in_=wo.ap())
            nc.sync.dma_start(out=bq_sb[:], in_=bq.ap())
            nc.sync.dma_start(out=bk_sb[:], in_=bk.ap())

            # ---------------- stage 1: projections ----------------
            HW = S // 2  # 1024-col halves of the sequence
            with (
                tc.tile_pool(name="xpool", bufs=2) as xpool,
                tc.tile_pool(name="psum1", bufs=2, space="PSUM") as psum1,
            ):
                # qhT = wq.T @ xqT (+bq), khT likewise
                for name, xT, w_sb, b_sb, outT in (
                    ("q", xqT, wq_sb, bq_sb, qhT),
                    ("k", xkT, wk_sb, bk_sb, khT),
                ):
                    for half in range(2):
                        x_sb = xpool.tile([128, 4, HW], F32, tag="x")
                        nc.sync.dma_start(
                            out=x_sb[:],
                            in_=xT.ap().rearrange("(c p) s -> p c s", p=128)[
                                :, :, half * HW : (half + 1) * HW
                            ],
                        )
                        ps = psum1.tile([D2, HW], F32, tag="ps1")
                        for ncx in range(HW // 512):
                            for cc in range(4):
                                nc.tensor.matmul(
                                    ps[:, ncx * 512 : (ncx + 1) * 512],
                                    w_sb[:, cc, :],
                                    x_sb[:, cc, ncx * 512 : (ncx + 1) * 512],
                                    start=(cc == 0),
                                    stop=(cc == 3),
                                )
                        nc.vector.tensor_scalar_add(
                            outT[:, half * HW : (half + 1) * HW], ps[:], b_sb[:]
                        )
                # vh[st block] = (xv @ wv) rows; bias folded on host
                for half in range(2):
                    x_sb = xpool.tile([128, 4, HW], F32, tag="x")
                    nc.sync.dma_start(
                        out=x_sb[:],
                        in_=xvT.ap().rearrange("(c p) s -> p c s", p=128)[
                            :, :, half * HW : (half + 1) * HW
                        ],
                    )
                    ps = psum1.tile([128, HW], F32, tag="ps1")
                    for sti in range(HW // 128):
                        for cc in range(4):
                            nc.tensor.matmul(
                                ps[:, sti * 128 : (sti + 1) * 128],
                                x_sb[:, cc, sti * 128 : (sti + 1) * 128],
                                wv_sb[:, cc, :],
                                start=(cc == 0),
                                stop=(cc == 3),
                            )
                    nc.vector.tensor_copy(
                        out=vh[:, half * HW : (half + 1) * HW], in_=ps[:]
                    )

            # ---------------- stage 2: attention ----------------
            with (
                tc.tile_pool(name="psum_p", bufs=1, space="PSUM") as pool_p,
                tc.tile_pool(name="psum_pt", bufs=1, space="PSUM") as pool_pt,
                tc.tile_pool(name="psum_av", bufs=2, space="PSUM") as pool_av,
                tc.tile_pool(name="Ppool", bufs=3) as Ppool,
                tc.tile_pool(name="stpool", bufs=2) as stpool,
                tc.tile_pool(name="PTpool", bufs=1) as PTpool,
                tc.tile_pool(name="sumpool", bufs=4) as sumpool,
            ):
                for h in range(HPC):
                    hs = slice(h * DEP, (h + 1) * DEP)
                    PT = PTpool.tile([128, NT, S], F16, tag="PT")
                    for t in range(NT):
                        # ---- P stream (q-tile t): attn rows out ----
                        ps_p = pool_p.tile([128, S], F32, tag="pp")
                        for ncx in range(4):
                            nc.tensor.matmul(
                                ps_p[:, ncx * 512 : (ncx + 1) * 512],
                                qhT[hs, t * 128 : (t + 1) * 128],
                                khT[hs, ncx * 512 : (ncx + 1) * 512],
                                start=True,
                                stop=True,
                            )
                        P_sb = Ppool.tile([128, S], F32, tag="P")
                        sums = sumpool.tile([128, 1], F32, tag="sums")
                        nc.scalar.activation(
                            P_sb[:],
                            ps_p[:],
                            mybir.ActivationFunctionType.Exp,
                            scale=float(scale),
                            accum_out=sums[:],
                        )
                        rc = recip[:, h * NT + t : h * NT + t + 1]
                        nc.vector.reciprocal(rc, sums[:])
                        nc.vector.tensor_scalar_mul(P_sb[:], P_sb[:], rc)
                        nc.sync.dma_start(
                            out=attn_out[h, t * 128 : (t + 1) * 128, :], in_=P_sb[:]
                        )
                        # ---- PT stream (k-tile t): transposed probs ----
                        st_sb = stpool.tile([128, S], F32, tag="st")
                        for half in range(2):
                            ps_t = pool_pt.tile([128, 1024], F32, tag="pt")
                            for ncx in range(2):
                                nc.tensor.matmul(
                                    ps_t[:, ncx * 512 : (ncx + 1) * 512],
                                    khT[hs, t * 128 : (t + 1) * 128],
                                    qhT[hs, half * 1024 + ncx * 512 : half * 1024 + (ncx + 1) * 512],
                                    start=True,
                                    stop=True,
                                )
                            nc.vector.tensor_copy(
                                out=st_sb[:, half * 1024 : (half + 1) * 1024],
                                in_=ps_t[:],
                            )
                        nc.scalar.activation(
                            PT[:, t, :],
                            st_sb[:],
                            mybir.ActivationFunctionType.Exp,
                            scale=float(scale),
                        )
                    # ---- AV: ctxT[h] += vh_h[kt].T @ PT[kt] over kt ----
                    for qc in range(4):
                        ps_av = pool_av.tile([DEP, 512], F32, tag="av")
                        for kt in range(NT):
                            nc.tensor.matmul(
                                ps_av[:],
                                vh[:, kt * 128 + h * DEP : kt * 128 + (h + 1) * DEP],
                                PT[:, kt, qc * 512 : (qc + 1) * 512],
                                start=(kt == 0),
                                stop=(kt == NT - 1),
                            )
                        nc.vector.tensor_copy(
                            out=ctxT[hs, qc * 512 : (qc + 1) * 512], in_=ps_av[:]
                        )

            # ---------------- stage 3: output projection ----------------
            with (
                tc.tile_pool(name="psum_o", bufs=4, space="PSUM") as pool_o,
                tc.tile_pool(name="opool", bufs=3) as opool,
            ):
                for t in range(NT):
                    acc = None
                    for h in range(HPC):
                        hs = slice(h * DEP, (h + 1) * DEP)
                        ps_o = pool_o.tile([128, D], F32, tag="po")
                        nc.tensor.matmul(
                            ps_o[:],
                            ctxT[hs, t * 128 : (t + 1) * 128],
                            wo_sb[hs, :],
                            start=True,
                            stop=True,
                        )
                        tmp = opool.tile([128, D], F32, tag=f"otmp{h}")
                        nc.vector.tensor_scalar_mul(
                            tmp[:], ps_o[:], recip[:, h * NT + t : h * NT + t + 1]
                        )
                        if acc is None:
                            acc = tmp
                        else:
                            nc.vector.tensor_add(acc[:], acc[:], tmp[:])
                    nc.sync.dma_start(
                        out=out_partial[t * 128 : (t + 1) * 128, :], in_=acc[:]
                    )

    nc.finalize()
    return nc


def kernel(q, k, v, wq_w, wq_b, wk_w, wk_b, wv_w, wv_b, wo_w, wo_b, _profile=False):
    global _CACHED_NC
    q = np.asarray(q, np.float32)
    k = np.asarray(k, np.float32)
    v = np.asarray(v, np.float32)
    wq_w = np.asarray(wq_w, np.float32)
    wk_w = np.asarray(wk_w, np.float32)
    wv_w = np.asarray(wv_w, np.float32)
    wo_w = np.asarray(wo_w, np.float32)

    if _CACHED_NC is None:
        _CACHED_NC = _build_nc()
    nc = _CACHED_NC

    xT = {}
    for b in range(B):
        xT[("q", b)] = np.ascontiguousarray(q[b].T)
        xT[("k", b)] = np.ascontiguousarray(k[b].T)
        xT[("v", b)] = np.ascontiguousarray(v[b].T)

    in_maps = []
    for c in range(N_CORES):
        b, hp = divmod(c, 4)
        cs = slice(hp * D2, (hp + 1) * D2)
        in_maps.append(
            {
                "xqT": xT[("q", b)],
                "xkT": xT[("k", b)],
                "xvT": xT[("v", b)],
                "wq": np.ascontiguousarray(wq_w[:, cs]),
                "wk": np.ascontiguousarray(wk_w[:, cs]),
                "wv": np.ascontiguousarray(wv_w[:, cs]),
                "wo": np.ascontiguousarray(wo_w[cs, :]),
                "bq": np.ascontiguousarray(np.asarray(wq_b, np.float32)[cs, None]),
                "bk": np.ascontiguousarray(np.asarray(wk_b, np.float32)[cs, None]),
            }
        )

    kwargs = {}
    if _profile:
        import os

        os.makedirs("/tmp/bass_trace", exist_ok=True)
        kwargs = {"trace": True, "tmpdir": "/tmp/bass_trace"}
    res = run_bass_kernel_spmd(nc, in_maps, list(range(N_CORES)), **kwargs)

    attn = np.empty((B, H, S, S), np.float32)
    out = np.zeros((B, S, D), np.float32)
    for c in range(N_CORES):
        b, hp = divmod(c, 4)
        attn[b, 2 * hp : 2 * hp + 2] = res.results[c]["attn_out"]
        out[b] += res.results[c]["out_partial"]
    # fold v/o biases: softmax rows sum to 1 -> ctx += wv_b, out += wv_b@wo + wo_b
    out += (
        np.asarray(wv_b, np.float32) @ wo_w + np.asarray(wo_b, np.float32)
    )[None, None, :]

    if _profile:
        return (out, attn), res
    return out, attn
